# revision 38
# baseline (speedup 1.0000x reference)
"""GAT/GCN message-passing layer on 8 Trainium2 NeuronCores.

Math: per query node i the reference computes
    e[i,j] = f_src[i] + f_dst[j]   (masked by Ahat>0), attn = softmax_j, then
    out = relu(attn @ h_prime).
f_src[i] is constant along the softmax axis so it cancels; with g = exp(f_dst)
the layer collapses to one GEMM over the adjacency:
    out = relu( (Ahat @ [g*h' | g])[:, :256] / (Ahat @ [g*h' | g])[:, 256:] )
with h' = node_feats @ w and f_dst = node_feats @ (w @ w_a @ a[2:4]).

Sharding: 1D row partition of query nodes; each core owns 1024 output rows and
keeps its [8192, 1024] adjacency slice pinned in SBUF as fp8 (binary matrix ->
lossless, half the HBM bytes).  w/u/node_feats^T are replicated; every core
recomputes the B = [g*h' | g] panel locally (an 8-way AllGather of the 4.2MB
panel would cost ~19us on the RMTV/D2D rings vs 14us of local PE recompute).

The B panel must stay bf16: quantizing it to fp8e4m3 measures maxrel 2.9e-2
against the f64 oracle (> the 2e-2 gate) because the softmax here is diffuse
(top weight ~2%), so outputs are ~82-term averages whose magnitude shrinks by
sqrt(deg) while fp8 noise only shrinks by the same factor - no headroom.  So
the main GEMM runs mixed fp8(A, stationary) x bf16(B, moving) at the bf16
streaming rate: 1 column/cycle, 512 MMs x 257 cols + 128 prefix MMs ~= 70us
of PE at 2.4 GHz.  The kernel is PE-bound; everything else hides behind it.

Key scheduling tricks (v2, from perfetto analysis of the 104.8us version):
  * HAM warmup: the PE clock sits at 1.2 GHz (K=4/8) until the activity
    monitor sees ~3.4us of sustained matmuls.  The old kernel's early MMs were
    DMA-paced and sparse, so it ran HALF CLOCK until t=33us (~11us wasted).
    A spin of junk matmuls on a zeroed SBUF tile starts the moment the
    framework preamble ends, latching full clock before real data arrives.
  * B-panel prep off the critical path: exp stays on ACT but the [128,256]
    h'*g multiply moves to the otherwise-idle DVE (265ns vs 560ns on ACT);
    the per-4-block g-column copies ride the same DVE chain so each main
    matmul's Bp dependency is a single DVE semaphore tick.  The old all-ACT
    chain (~780ns/j) out-paced the main stream's 660ns/j consumption and
    stalled the stream end.
  * All DRAM tensors are pre-tiled SBUF images (partition-major), so every
    DMA is 128 contiguous descriptors; params and adjacency are split into
    13 chunks whose descriptor generations pipeline on the SYNC sequencer
    while transfers run on parallel queues (loads never gate the PE after
    the first prefix block).
  * The prefix (h' panel) is interleaved with the main GEMM two blocks at a
    time (prefix 2k, 2k+1, then main 2k-4, 2k-3): h' borrows PSUM banks 6/7
    while main i-blocks 0-5 accumulate in banks 0-5.  i-blocks 6/7 are
    backfilled bank-major after the prefix finishes (all of A is resident, so
    the backfill is pure PE work, and bank 6's epilogue/store overlap bank
    7's matmuls).
  * walrus accepts only ONE sync wait per instruction, so each instruction's
    cross-engine deps stay on a single engine: main matmuls wait only on the
    DVE chain, DMA first-touches are absorbed by PE nops at chunk boundaries,
    and the output stores go through gpsimd whose DMA queues carry no load
    traffic.  A tail funnel of SYNC nops keeps the kernel-exit drain to a
    single wait per proc.
"""

import sys

import ml_dtypes
import numpy as np

sys.path.insert(0, "/opt/trn_rl_repo")

import concourse.bass as bass  # noqa: E402
import concourse.tile as tile  # noqa: E402
from concourse import mybir  # noqa: E402
from concourse.bass_utils import run_bass_kernel_spmd  # noqa: E402
from concourse.tile import add_dep_helper  # noqa: E402

N = 8192
F = 256  # in_features == out_features
FE = F + 1  # h' columns + the g column
NCORES = 8
ROWS = N // NCORES  # 1024 output rows per core
P = 128
NJ = N // P  # 64 contraction blocks
NI = ROWS // P  # 8 output-row blocks per core

BF = mybir.dt.bfloat16
F8 = mybir.dt.float8e4
F32 = mybir.dt.float32

# node_feats^T / adjacency chunk splits, in j-blocks.  Each dma_start
# sustains only ~90GB/s (ring-shared aggregate across in-flight transfers),
# so the front chunks are small for latency and the tail chunks ~1MB so
# several stay in flight.
NSPLIT = [0, 2, 8, 16, 32, 48, 64]
ASPLIT = [0, 2, 8, 20, 34, 48, 64]
# The warmup spin covers the first chunks' DMA latency so the real stream
# starts warm; it should end right as the first param chunk lands.
WARMUP_MMS = 26
FILLER_KS = 0

_CACHE = {}


def _build():
    nc = bass.Bass(
        "TRN2",
        target_bir_lowering=False,
        debug=False,
        enable_asserts=True,
        num_devices=NCORES,
    )
    # pre-tiled images (partition-major; see _prep_inputs)
    aT = nc.dram_tensor("aT", [P, NJ, ROWS], F8, kind="ExternalInput").ap()
    pw = nc.dram_tensor("pw", [P, 2, FE], BF, kind="ExternalInput").ap()
    nfc = tuple(
        nc.dram_tensor(
            f"nf{c}", [P, 2, (NSPLIT[c + 1] - NSPLIT[c]) * P], BF, kind="ExternalInput"
        ).ap()
        for c in range(len(NSPLIT) - 1)
    )
    out = nc.dram_tensor("out", [P, NI, F], F32, kind="ExternalOutput").ap()

    with tile.TileContext(nc) as tc:
        _body(tc, aT, pw, nfc, out)
    return nc


def _body(tc, aT, pw, nfc, out):
    nc = tc.nc
    Exp = mybir.ActivationFunctionType.Exp
    Relu = mybir.ActivationFunctionType.Relu
    NCH = len(NSPLIT) - 1

    with (
        tc.tile_pool(name="consts", bufs=1) as consts,
        tc.tile_pool(name="rpool", bufs=8) as rpool,
        tc.tile_pool(name="psum", bufs=1, space="PSUM") as psum,
    ):
        # ---- SBUF tiles ----------------------------------------------------
        pw_sb = consts.tile([P, 2, FE], BF, tag="pw", name="pw")
        nf_sb = [
            consts.tile(
                [P, 2, (NSPLIT[c + 1] - NSPLIT[c]) * P],
                BF,
                tag=f"nf{c}",
                name=f"nf{c}",
            )
            for c in range(NCH)
        ]
        aT_sb = consts.tile([P, NJ, ROWS], F8, tag="aT")
        Bp = consts.tile([P, NJ, FE], BF, tag="Bp")  # [g*h' | g] panel
        G = consts.tile([P, NJ], F32, tag="G")  # g = exp(f_dst)
        otile = consts.tile([P, NI * F], F32, tag="o")
        junk = consts.tile([P, P], BF, tag="junk")  # HAM warmup operand

        def nfT(j, kb):
            """SBUF [128, 128] lhsT view of node_feats^T block j, k-half kb."""
            for c in range(NCH - 1, -1, -1):
                if j >= NSPLIT[c]:
                    lo = NSPLIT[c]
                    return nf_sb[c][:, kb, (j - lo) * P : (j - lo + 1) * P]

        # ---- loads ----------------------------------------------------------
        # Two latency-critical front chunks (nf1, a0) ride the otherwise
        # idle ACT HW-DGE ring so their descriptor generations run in
        # parallel with SYNC's; everything else stays on SYNC's ring (more
        # ACT gens would delay the exp chain, which shares the ACT FIFO).
        # Loads carry no data deps so a queue-reuse wait is their only sync.
        # gpsimd's SWDGE queues stay virgin for the output stores.
        prev_ring = {"s": None, "a": None}
        ring_eng = {"s": nc.sync, "a": nc.scalar}

        def load(r, dst, src):
            d = ring_eng[r].dma_start(dst, src)
            if prev_ring[r] is not None:
                add_dep_helper(d.ins, prev_ring[r].ins, sync=False, reason="dma order")
            prev_ring[r] = d
            return d

        def a_load(r, c):
            lo, hi = ASPLIT[c], ASPLIT[c + 1]
            return load(r, aT_sb[:, lo:hi, :], aT[:, lo:hi, :])

        def nf_load(r, c):
            return load(r, nf_sb[c][:], nfc[c][:])

        nf_dma = [None] * NCH
        a_dma = [None] * (len(ASPLIT) - 1)
        pw_dma = load("s", pw_sb[:], pw[:])
        nf_dma[0] = nf_load("s", 0)
        nf_dma[1] = nf_load("a", 1)
        a_dma[0] = a_load("a", 0)
        ni, ai = 2, 1
        while ni < NCH or ai < len(a_dma):
            if ni < NCH:
                nf_dma[ni] = nf_load("s", ni)
                ni += 1
            if ai < len(a_dma):
                a_dma[ai] = a_load("s", ai)
                ai += 1

        # ---- PSUM accumulators ----------------------------------------------
        # full-bank tiles: cols 0:257 hold the accumulation, the top 128
        # cols of bank 7 are a scratch target for warmup/filler junk matmuls
        # (has_written is per element, so disjoint columns don't interact)
        acc = [
            psum.tile([P, 512], F32, tag=f"acc{i}", name=f"acc{i}") for i in range(NI)
        ]
        # prefix h' borrows banks 6/7 (i-blocks 6/7 are backfilled later)
        hp = [acc[6], acc[7]]
        junk_ps = acc[7][:, 384:512]

        # ---- HAM warmup spin -------------------------------------------------
        # junk matmuls with no DMA dependency keep the PE busy from the end of
        # the framework preamble so the HAM clock gate opens (1.2 -> 2.4 GHz)
        # before the real stream begins.
        nc.gpsimd.memset(junk[:], 0.0)
        for _ in range(WARMUP_MMS):
            nc.tensor.matmul(
                junk_ps[:],
                lhsT=junk[:],
                rhs=junk[:],
                start=True,
                stop=True,
                skip_group_check=True,
            )

        # first-touch absorbers: a PE nop waits on the DMA so the matmul that
        # follows needs only its compute-engine wait
        def pe_gate(dma):
            nop = nc.tensor.nop(nofuse=True, hint="dma_gate")
            add_dep_helper(nop.ins, dma.ins, reason="dma gate")
            return nop

        prev_act = None
        prev_dve = None
        last_mm = None

        def prefix(j):
            nonlocal prev_act, prev_dve, last_mm
            h = hp[j % 2]
            for kb in range(2):
                last_mm = nc.tensor.matmul(
                    h[:, 0:FE],
                    lhsT=nfT(j, kb),
                    rhs=pw_sb[:, kb, :],
                    start=(kb == 0),
                    stop=(kb == 1),
                )
            # G[:, j] = exp(f_dst) on ACT; takes the single PE wait for this
            # bank.  The DVE ops below then only need the ACT tick (PE write
            # is ordered-before transitively).
            # G[:, j] = exp(f_dst); takes the single PE wait for this bank so
            # the Bp write below needs none (dominated, elided).  The whole
            # Bp-prep chain stays on ACT: tile serializes PSUM readers across
            # engines, so a second reader engine would pick up a second sync
            # wait, which walrus rejects.  ACT's 820ns/j is under the PE's
            # 880ns/j consumption rate, so the chain never paces the stream.
            ex = nc.scalar.activation(G[:, j : j + 1], h[:, F : F + 1], Exp)
            if prev_act is not None:
                add_dep_helper(ex.ins, prev_act.ins, sync=False, reason="act order")
            # Bp[j] = bf16(h' * g)
            bp = nc.scalar.mul(Bp[:, j, 0:F], h[:, 0:F], G[:, j : j + 1])
            add_dep_helper(bp.ins, ex.ins, sync=False, reason="act order")
            prev_act = bp
            if j % 4 == 3:
                # drop this 4-group's g columns into Bp (one strided cast-copy)
                j0 = j - 3
                gh = nc.scalar.copy(Bp[:, j0 : j + 1, F], G[:, j0 : j + 1])
                add_dep_helper(gh.ins, prev_act.ins, sync=False, reason="act order")
                prev_act = gh

        def main_block(j, ilist, start, stop):
            nonlocal last_mm
            for i in ilist:
                last_mm = nc.tensor.matmul(
                    acc[i][:, 0:FE],
                    lhsT=aT_sb[:, j, i * P : (i + 1) * P],
                    rhs=Bp[:, j, :],
                    start=start,
                    stop=stop,
                )

        # ---- interleaved prefix + main stream --------------------------------
        a_gate = {ASPLIT[c]: a_dma[c] for c in range(len(a_dma))}
        nf_gate = {NSPLIT[c]: nf_dma[c] for c in range(1, NCH)}
        pe_gate(pw_dma)
        pe_gate(nf_dma[0])

        def filler(n):
            # dependency-free junk matmuls keep the PE's activity monitor
            # latched through early DMA waits (a >~2us PE idle re-throttles
            # the clock to 1.2 GHz for several microseconds).  Emitted BEFORE
            # the instructions that wait, since the PE queue is FIFO.
            for _ in range(n):
                nc.tensor.matmul(
                    junk_ps[:],
                    lhsT=junk[:],
                    rhs=junk[:],
                    start=True,
                    stop=True,
                    skip_group_check=True,
                )

        for k in range(NJ // 2):
            if 0 < k <= FILLER_KS:
                filler(4)
            for j in (2 * k, 2 * k + 1):
                if j in nf_gate:
                    pe_gate(nf_gate[j])
                prefix(j)
            if 0 < k <= FILLER_KS:
                filler(2)
            for jm in (2 * k - 4, 2 * k - 3):
                if jm < 0:
                    continue
                if jm in a_gate:
                    pe_gate(a_gate[jm])
                main_block(jm, range(6), start=(jm == 0), stop=False)
        del filler
        for jm in range(NJ - 4, NJ):
            main_block(jm, range(6), start=False, stop=(jm == NJ - 1))
        # backfill i-blocks 7/6 (banks free once the prefix drained);
        # bank-major so bank 7's epilogue and store overlap bank 6's matmuls,
        # leaving only bank 6's short DVE chain + store exposed at the end
        for i in (7, 6):
            for j in range(NJ):
                main_block(j, (i,), start=(j == 0), stop=(j == NJ - 1))

        # ---- epilogue: out[i] = relu(acc[i][:, :F] / acc[i][:, F]) ----------
        # banks 0..5 finish at the end of the interleaved stream and drain on
        # ACT while the backfill matmuls still run; banks 6/7 drain via DVE.
        stores = []
        banksA = list(range(6))
        denomA = rpool.tile([P, len(banksA)], F32, tag="denomA")
        denom_last = None
        for k, i in enumerate(banksA):
            dc = nc.scalar.copy(denomA[:, k : k + 1], acc[i][:, F : F + 1])
            if denom_last is not None:
                add_dep_helper(dc.ins, denom_last.ins, sync=False, reason="act order")
            denom_last = dc
        recipA = rpool.tile([P, len(banksA)], F32, tag="recipA")
        nc.vector.reciprocal(recipA[:], denomA[:])
        # sacrificial ACT read absorbs the DVE tick for the six fused relus
        sacA = rpool.tile([P, len(banksA)], F32, tag="sacA")
        sa = nc.scalar.copy(sacA[:], recipA[:])
        add_dep_helper(sa.ins, denom_last.ins, sync=False, reason="act order")
        last_relu = sa
        for k, i in enumerate(banksA):
            o = otile[:, i * F : (i + 1) * F]
            rl = nc.scalar.activation(
                o, acc[i][:, 0:F], Relu, scale=recipA[:, k : k + 1]
            )
            add_dep_helper(rl.ins, last_relu.ins, sync=False, reason="act order")
            last_relu = rl
        # stores on gpsimd's virgin SWDGE queues; a gpsimd nop absorbs the
        # ACT data dep so the store carries a single wait
        gnop = nc.gpsimd.nop(nofuse=True, hint="storeA_gate")
        add_dep_helper(gnop.ins, last_relu.ins, reason="storeA gate")
        stores.append(nc.gpsimd.dma_start(out[:, 0:6, :], otile[:, 0 : 6 * F]))
        add_dep_helper(stores[-1].ins, gnop.ins, sync=False, reason="after gate")

        # banks 7/6 (backfill): DVE path, one chain per bank so bank 7's
        # store overlaps bank 6's backfill matmuls; mult+max fuse into one
        # tensor_scalar.
        Mult = mybir.AluOpType.mult
        Max = mybir.AluOpType.max
        denomB = rpool.tile([P, 2], F32, tag="denomB")
        recipB = rpool.tile([P, 2], F32, tag="recipB")
        last_dve = None
        for k, i in enumerate([7, 6]):
            # DVE-only chain: the 1-col denominator copy takes the PE wait
            nc.vector.tensor_copy(denomB[:, k : k + 1], acc[i][:, F : F + 1])
            nc.vector.reciprocal(recipB[:, k : k + 1], denomB[:, k : k + 1])
            o = otile[:, i * F : (i + 1) * F]
            last_dve = nc.vector.tensor_scalar(
                o, acc[i][:, 0:F], recipB[:, k : k + 1], 0.0, Mult, Max
            )
            gnop = nc.gpsimd.nop(nofuse=True, hint=f"store{i}_gate")
            add_dep_helper(gnop.ins, last_dve.ins, reason=f"store{i} gate")
            stores.append(
                nc.gpsimd.dma_start(out[:, i : i + 1, :], otile[:, i * F : (i + 1) * F])
            )
            add_dep_helper(stores[-1].ins, gnop.ins, sync=False, reason="after gate")

        # funnel every proc's final tick into SP via single-wait nops so the
        # kernel-tail drain has nothing left to wait on (every DMA queue's
        # final count included, else the drain aggregates 10+ waits)
        for dep in [
            pw_dma,
            *nf_dma,
            *a_dma,
            *stores,
            last_mm,
            last_relu,
            last_dve,
            prev_act,
        ]:
            nop = nc.sync.nop(nofuse=True, hint="tail_funnel")
            add_dep_helper(nop.ins, dep.ins, reason="tail funnel")


def _prep_inputs(node_feats, Ahat, w, w_a, a):
    node_feats = np.asarray(node_feats, dtype=np.float32)
    Ahat = np.asarray(Ahat, dtype=np.float32)
    w = np.asarray(w, dtype=np.float32)
    w_a = np.asarray(w_a, dtype=np.float32)
    a = np.asarray(a, dtype=np.float32)

    u = w @ (w_a @ a[2:4])  # [256, 1]
    # wext [256, 257] -> partition-major image [128, 2, 257]
    Mw = np.concatenate([w, u], axis=1).astype(ml_dtypes.bfloat16)
    pw_img = np.ascontiguousarray(Mw.reshape(2, P, FE).transpose(1, 0, 2))
    # node_feats^T [256, 8192] -> [128, 2, 8192], chunked along j
    nfT_img = node_feats.T.astype(ml_dtypes.bfloat16).reshape(2, P, N).transpose(1, 0, 2)
    nf_chunks = {
        f"nf{c}": np.ascontiguousarray(
            nfT_img[:, :, NSPLIT[c] * P : NSPLIT[c + 1] * P]
        )
        for c in range(len(NSPLIT) - 1)
    }

    in_maps = []
    for c in range(NCORES):
        aT_c = Ahat[c * ROWS : (c + 1) * ROWS, :].T  # [8192, 1024]
        aT_img = np.ascontiguousarray(
            aT_c.reshape(NJ, P, ROWS).transpose(1, 0, 2).astype(ml_dtypes.float8_e4m3)
        )
        in_maps.append({"aT": aT_img, "pw": pw_img, **nf_chunks})
    return in_maps


def _run(inputs, trace=False, **kwargs):
    if "nc" not in _CACHE:
        _CACHE["nc"] = _build()
    nc = _CACHE["nc"]
    in_maps = _prep_inputs(**inputs)
    res = run_bass_kernel_spmd(
        nc, in_maps, core_ids=list(range(NCORES)), trace=trace, **kwargs
    )
    # out image [128, 8, 256] -> rows (i*128 + p)
    full = np.concatenate(
        [
            res.results[c]["out"].transpose(1, 0, 2).reshape(ROWS, F)
            for c in range(NCORES)
        ],
        axis=0,
    )
    return full, res


def kernel(**inputs) -> np.ndarray:
    out, _ = _run(inputs, trace=False)
    return out


# revision 40
# speedup vs baseline: 1.0373x; 1.0373x over previous
"""GAT/GCN message-passing layer on 8 Trainium2 NeuronCores.

Math: per query node i the reference computes
    e[i,j] = f_src[i] + f_dst[j]   (masked by Ahat>0), attn = softmax_j, then
    out = relu(attn @ h_prime).
f_src[i] is constant along the softmax axis so it cancels; with g = exp(f_dst)
the layer collapses to one GEMM over the adjacency:
    out = relu( (Ahat @ [g*h' | g])[:, :256] / (Ahat @ [g*h' | g])[:, 256:] )
with h' = node_feats @ w and f_dst = node_feats @ (w @ w_a @ a[2:4]).

Sharding: 1D row partition of query nodes; each core owns 1024 output rows and
keeps its [8192, 1024] adjacency slice pinned in SBUF as fp8 (binary matrix ->
lossless, half the HBM bytes).  w/u/node_feats^T are replicated; every core
recomputes the B = [g*h' | g] panel locally (an 8-way AllGather of the 4.2MB
panel would cost ~19us on the RMTV/D2D rings vs 14us of local PE recompute).

The B panel must stay bf16: quantizing it to fp8e4m3 measures maxrel 2.9e-2
against the f64 oracle (> the 2e-2 gate) because the softmax here is diffuse
(top weight ~2%), so outputs are ~82-term averages whose magnitude shrinks by
sqrt(deg) while fp8 noise only shrinks by the same factor - no headroom.  So
the main GEMM runs mixed fp8(A, stationary) x bf16(B, moving) at the bf16
streaming rate: 1 column/cycle, 512 MMs x 257 cols + 128 prefix MMs ~= 70us
of PE at 2.4 GHz.  The kernel is PE-bound; everything else hides behind it.

Key scheduling tricks (v2, from perfetto analysis of the 104.8us version):
  * HAM warmup: the PE clock sits at 1.2 GHz (K=4/8) until the activity
    monitor sees ~3.4us of sustained matmuls.  The old kernel's early MMs were
    DMA-paced and sparse, so it ran HALF CLOCK until t=33us (~11us wasted).
    A spin of junk matmuls on a zeroed SBUF tile starts the moment the
    framework preamble ends, latching full clock before real data arrives.
  * B-panel prep off the critical path: exp stays on ACT but the [128,256]
    h'*g multiply moves to the otherwise-idle DVE (265ns vs 560ns on ACT);
    the per-4-block g-column copies ride the same DVE chain so each main
    matmul's Bp dependency is a single DVE semaphore tick.  The old all-ACT
    chain (~780ns/j) out-paced the main stream's 660ns/j consumption and
    stalled the stream end.
  * All DRAM tensors are pre-tiled SBUF images (partition-major), so every
    DMA is 128 contiguous descriptors; params and adjacency are split into
    13 chunks whose descriptor generations pipeline on the SYNC sequencer
    while transfers run on parallel queues (loads never gate the PE after
    the first prefix block).
  * The prefix (h' panel) is interleaved with the main GEMM two blocks at a
    time (prefix 2k, 2k+1, then main 2k-4, 2k-3): h' borrows PSUM banks 6/7
    while main i-blocks 0-5 accumulate in banks 0-5.  i-blocks 6/7 are
    backfilled bank-major after the prefix finishes (all of A is resident, so
    the backfill is pure PE work, and bank 6's epilogue/store overlap bank
    7's matmuls).
  * walrus accepts only ONE sync wait per instruction, so each instruction's
    cross-engine deps stay on a single engine: main matmuls wait only on the
    DVE chain, DMA first-touches are absorbed by PE nops at chunk boundaries,
    and the output stores go through gpsimd whose DMA queues carry no load
    traffic.  A tail funnel of SYNC nops keeps the kernel-exit drain to a
    single wait per proc.
"""

import sys

import ml_dtypes
import numpy as np

sys.path.insert(0, "/opt/trn_rl_repo")

import concourse.bass as bass  # noqa: E402
import concourse.tile as tile  # noqa: E402
from concourse import mybir  # noqa: E402
from concourse.bass_utils import run_bass_kernel_spmd  # noqa: E402
from concourse.tile import add_dep_helper  # noqa: E402

N = 8192
F = 256  # in_features == out_features
FE = F + 1  # h' columns + the g column
NCORES = 8
ROWS = N // NCORES  # 1024 output rows per core
P = 128
NJ = N // P  # 64 contraction blocks
NI = ROWS // P  # 8 output-row blocks per core

BF = mybir.dt.bfloat16
F8 = mybir.dt.float8e4
F32 = mybir.dt.float32

# node_feats^T / adjacency chunk splits, in j-blocks.  Each dma_start
# sustains only ~90GB/s (ring-shared aggregate across in-flight transfers),
# so the front chunks are small for latency and the tail chunks ~1MB so
# several stay in flight.
NSPLIT = [0, 2, 8, 16, 32, 48, 64]
ASPLIT = [0, 2, 8, 20, 34, 48, 64]
# The warmup spin covers the first chunks' DMA latency so the real stream
# starts warm; it should end right as the first param chunk lands.
WARMUP_MMS = 22
FILLER_KS = 0

_CACHE = {}


def _build():
    nc = bass.Bass(
        "TRN2",
        target_bir_lowering=False,
        debug=False,
        enable_asserts=True,
        num_devices=NCORES,
    )
    # pre-tiled images (partition-major; see _prep_inputs)
    aT = nc.dram_tensor("aT", [P, NJ, ROWS], F8, kind="ExternalInput").ap()
    pw = nc.dram_tensor("pw", [P, 2, FE], BF, kind="ExternalInput").ap()
    nfc = tuple(
        nc.dram_tensor(
            f"nf{c}", [P, 2, (NSPLIT[c + 1] - NSPLIT[c]) * P], BF, kind="ExternalInput"
        ).ap()
        for c in range(len(NSPLIT) - 1)
    )
    out = nc.dram_tensor("out", [P, NI, F], F32, kind="ExternalOutput").ap()

    with tile.TileContext(nc) as tc:
        _body(tc, aT, pw, nfc, out)
    return nc


def _body(tc, aT, pw, nfc, out):
    nc = tc.nc
    Exp = mybir.ActivationFunctionType.Exp
    Relu = mybir.ActivationFunctionType.Relu
    NCH = len(NSPLIT) - 1

    with (
        tc.tile_pool(name="consts", bufs=1) as consts,
        tc.tile_pool(name="rpool", bufs=8) as rpool,
        tc.tile_pool(name="psum", bufs=1, space="PSUM") as psum,
    ):
        # ---- SBUF tiles ----------------------------------------------------
        pw_sb = consts.tile([P, 2, FE], BF, tag="pw", name="pw")
        nf_sb = [
            consts.tile(
                [P, 2, (NSPLIT[c + 1] - NSPLIT[c]) * P],
                BF,
                tag=f"nf{c}",
                name=f"nf{c}",
            )
            for c in range(NCH)
        ]
        aT_sb = consts.tile([P, NJ, ROWS], F8, tag="aT")
        Bp = consts.tile([P, NJ, FE], BF, tag="Bp")  # [g*h' | g] panel
        G = consts.tile([P, NJ], F32, tag="G")  # g = exp(f_dst)
        otile = consts.tile([P, NI * F], F32, tag="o")
        junk = consts.tile([P, P], BF, tag="junk")  # HAM warmup operand

        def nfT(j, kb):
            """SBUF [128, 128] lhsT view of node_feats^T block j, k-half kb."""
            for c in range(NCH - 1, -1, -1):
                if j >= NSPLIT[c]:
                    lo = NSPLIT[c]
                    return nf_sb[c][:, kb, (j - lo) * P : (j - lo + 1) * P]

        # ---- loads ----------------------------------------------------------
        # Two latency-critical front chunks (nf1, a0) ride the otherwise
        # idle ACT HW-DGE ring so their descriptor generations run in
        # parallel with SYNC's; everything else stays on SYNC's ring (more
        # ACT gens would delay the exp chain, which shares the ACT FIFO).
        # Loads carry no data deps so a queue-reuse wait is their only sync.
        # gpsimd's SWDGE queues stay virgin for the output stores.
        prev_ring = {"s": None, "a": None}
        ring_eng = {"s": nc.sync, "a": nc.scalar}

        def load(r, dst, src):
            d = ring_eng[r].dma_start(dst, src)
            if prev_ring[r] is not None:
                add_dep_helper(d.ins, prev_ring[r].ins, sync=False, reason="dma order")
            prev_ring[r] = d
            return d

        def a_load(r, c):
            lo, hi = ASPLIT[c], ASPLIT[c + 1]
            return load(r, aT_sb[:, lo:hi, :], aT[:, lo:hi, :])

        def nf_load(r, c):
            return load(r, nf_sb[c][:], nfc[c][:])

        nf_dma = [None] * NCH
        a_dma = [None] * (len(ASPLIT) - 1)
        pw_dma = load("s", pw_sb[:], pw[:])
        ni, ai = 0, 0
        while ni < NCH or ai < len(a_dma):
            if ni < NCH:
                nf_dma[ni] = nf_load("s", ni)
                ni += 1
            if ai < len(a_dma):
                a_dma[ai] = a_load("s", ai)
                ai += 1

        # ---- PSUM accumulators ----------------------------------------------
        # full-bank tiles: cols 0:257 hold the accumulation, the top 128
        # cols of bank 7 are a scratch target for warmup/filler junk matmuls
        # (has_written is per element, so disjoint columns don't interact)
        acc = [
            psum.tile([P, 512], F32, tag=f"acc{i}", name=f"acc{i}") for i in range(NI)
        ]
        # prefix h' borrows banks 6/7 (i-blocks 6/7 are backfilled later)
        hp = [acc[6], acc[7]]
        junk_ps = acc[7][:, 384:512]

        # ---- HAM warmup spin -------------------------------------------------
        # junk matmuls with no DMA dependency keep the PE busy from the end of
        # the framework preamble so the HAM clock gate opens (1.2 -> 2.4 GHz)
        # before the real stream begins.
        nc.gpsimd.memset(junk[:], 0.0)
        for _ in range(WARMUP_MMS):
            nc.tensor.matmul(
                junk_ps[:],
                lhsT=junk[:],
                rhs=junk[:],
                start=True,
                stop=True,
                skip_group_check=True,
            )

        # first-touch absorbers: a PE nop waits on the DMA so the matmul that
        # follows needs only its compute-engine wait
        def pe_gate(dma):
            nop = nc.tensor.nop(nofuse=True, hint="dma_gate")
            add_dep_helper(nop.ins, dma.ins, reason="dma gate")
            return nop

        prev_act = None
        prev_dve = None
        last_mm = None

        def prefix(j):
            nonlocal prev_act, prev_dve, last_mm
            h = hp[j % 2]
            for kb in range(2):
                last_mm = nc.tensor.matmul(
                    h[:, 0:FE],
                    lhsT=nfT(j, kb),
                    rhs=pw_sb[:, kb, :],
                    start=(kb == 0),
                    stop=(kb == 1),
                )
            # G[:, j] = exp(f_dst) on ACT; takes the single PE wait for this
            # bank.  The DVE ops below then only need the ACT tick (PE write
            # is ordered-before transitively).
            # G[:, j] = exp(f_dst); takes the single PE wait for this bank so
            # the Bp write below needs none (dominated, elided).  The whole
            # Bp-prep chain stays on ACT: tile serializes PSUM readers across
            # engines, so a second reader engine would pick up a second sync
            # wait, which walrus rejects.  ACT's 820ns/j is under the PE's
            # 880ns/j consumption rate, so the chain never paces the stream.
            ex = nc.scalar.activation(G[:, j : j + 1], h[:, F : F + 1], Exp)
            if prev_act is not None:
                add_dep_helper(ex.ins, prev_act.ins, sync=False, reason="act order")
            # Bp[j] = bf16(h' * g)
            bp = nc.scalar.mul(Bp[:, j, 0:F], h[:, 0:F], G[:, j : j + 1])
            add_dep_helper(bp.ins, ex.ins, sync=False, reason="act order")
            prev_act = bp
            if j % 4 == 3:
                # drop this 4-group's g columns into Bp (one strided cast-copy)
                j0 = j - 3
                gh = nc.scalar.copy(Bp[:, j0 : j + 1, F], G[:, j0 : j + 1])
                add_dep_helper(gh.ins, prev_act.ins, sync=False, reason="act order")
                prev_act = gh

        def main_block(j, ilist, start, stop):
            nonlocal last_mm
            for i in ilist:
                last_mm = nc.tensor.matmul(
                    acc[i][:, 0:FE],
                    lhsT=aT_sb[:, j, i * P : (i + 1) * P],
                    rhs=Bp[:, j, :],
                    start=start,
                    stop=stop,
                )

        # ---- interleaved prefix + main stream --------------------------------
        a_gate = {ASPLIT[c]: a_dma[c] for c in range(len(a_dma))}
        nf_gate = {NSPLIT[c]: nf_dma[c] for c in range(1, NCH)}
        pe_gate(pw_dma)
        pe_gate(nf_dma[0])

        def filler(n):
            # dependency-free junk matmuls keep the PE's activity monitor
            # latched through early DMA waits (a >~2us PE idle re-throttles
            # the clock to 1.2 GHz for several microseconds).  Emitted BEFORE
            # the instructions that wait, since the PE queue is FIFO.
            for _ in range(n):
                nc.tensor.matmul(
                    junk_ps[:],
                    lhsT=junk[:],
                    rhs=junk[:],
                    start=True,
                    stop=True,
                    skip_group_check=True,
                )

        for k in range(NJ // 2):
            if 0 < k <= FILLER_KS:
                filler(4)
            for j in (2 * k, 2 * k + 1):
                if j in nf_gate:
                    pe_gate(nf_gate[j])
                prefix(j)
            if 0 < k <= FILLER_KS:
                filler(2)
            for jm in (2 * k - 4, 2 * k - 3):
                if jm < 0:
                    continue
                if jm in a_gate:
                    pe_gate(a_gate[jm])
                main_block(jm, range(6), start=(jm == 0), stop=False)
        del filler
        for jm in range(NJ - 4, NJ):
            main_block(jm, range(6), start=False, stop=(jm == NJ - 1))
        # backfill i-blocks 7/6 (banks free once the prefix drained);
        # bank-major so bank 7's epilogue and store overlap bank 6's matmuls,
        # leaving only bank 6's short DVE chain + store exposed at the end
        for i in (7, 6):
            for j in range(NJ):
                main_block(j, (i,), start=(j == 0), stop=(j == NJ - 1))

        # ---- epilogue: out[i] = relu(acc[i][:, :F] / acc[i][:, F]) ----------
        # banks 0..5 finish at the end of the interleaved stream and drain on
        # ACT while the backfill matmuls still run; banks 6/7 drain via DVE.
        stores = []
        banksA = list(range(6))
        denomA = rpool.tile([P, len(banksA)], F32, tag="denomA")
        denom_last = None
        for k, i in enumerate(banksA):
            dc = nc.scalar.copy(denomA[:, k : k + 1], acc[i][:, F : F + 1])
            if denom_last is not None:
                add_dep_helper(dc.ins, denom_last.ins, sync=False, reason="act order")
            denom_last = dc
        recipA = rpool.tile([P, len(banksA)], F32, tag="recipA")
        nc.vector.reciprocal(recipA[:], denomA[:])
        # sacrificial ACT read absorbs the DVE tick for the six fused relus
        sacA = rpool.tile([P, len(banksA)], F32, tag="sacA")
        sa = nc.scalar.copy(sacA[:], recipA[:])
        add_dep_helper(sa.ins, denom_last.ins, sync=False, reason="act order")
        last_relu = sa
        for k, i in enumerate(banksA):
            o = otile[:, i * F : (i + 1) * F]
            rl = nc.scalar.activation(
                o, acc[i][:, 0:F], Relu, scale=recipA[:, k : k + 1]
            )
            add_dep_helper(rl.ins, last_relu.ins, sync=False, reason="act order")
            last_relu = rl
        # stores on gpsimd's virgin SWDGE queues; a gpsimd nop absorbs the
        # ACT data dep so the store carries a single wait
        gnop = nc.gpsimd.nop(nofuse=True, hint="storeA_gate")
        add_dep_helper(gnop.ins, last_relu.ins, reason="storeA gate")
        stores.append(nc.gpsimd.dma_start(out[:, 0:6, :], otile[:, 0 : 6 * F]))
        add_dep_helper(stores[-1].ins, gnop.ins, sync=False, reason="after gate")

        # banks 7/6 (backfill): DVE path, one chain per bank so bank 7's
        # store overlaps bank 6's backfill matmuls; mult+max fuse into one
        # tensor_scalar.
        Mult = mybir.AluOpType.mult
        Max = mybir.AluOpType.max
        denomB = rpool.tile([P, 2], F32, tag="denomB")
        recipB = rpool.tile([P, 2], F32, tag="recipB")
        last_dve = None
        for k, i in enumerate([7, 6]):
            # DVE-only chain: the 1-col denominator copy takes the PE wait
            nc.vector.tensor_copy(denomB[:, k : k + 1], acc[i][:, F : F + 1])
            nc.vector.reciprocal(recipB[:, k : k + 1], denomB[:, k : k + 1])
            o = otile[:, i * F : (i + 1) * F]
            last_dve = nc.vector.tensor_scalar(
                o, acc[i][:, 0:F], recipB[:, k : k + 1], 0.0, Mult, Max
            )
            gnop = nc.gpsimd.nop(nofuse=True, hint=f"store{i}_gate")
            add_dep_helper(gnop.ins, last_dve.ins, reason=f"store{i} gate")
            stores.append(
                nc.gpsimd.dma_start(out[:, i : i + 1, :], otile[:, i * F : (i + 1) * F])
            )
            add_dep_helper(stores[-1].ins, gnop.ins, sync=False, reason="after gate")

        # funnel every proc's final tick into SP via single-wait nops so the
        # kernel-tail drain has nothing left to wait on (every DMA queue's
        # final count included, else the drain aggregates 10+ waits)
        for dep in [
            pw_dma,
            *nf_dma,
            *a_dma,
            *stores,
            last_mm,
            last_relu,
            last_dve,
            prev_act,
        ]:
            nop = nc.sync.nop(nofuse=True, hint="tail_funnel")
            add_dep_helper(nop.ins, dep.ins, reason="tail funnel")


def _prep_inputs(node_feats, Ahat, w, w_a, a):
    node_feats = np.asarray(node_feats, dtype=np.float32)
    Ahat = np.asarray(Ahat, dtype=np.float32)
    w = np.asarray(w, dtype=np.float32)
    w_a = np.asarray(w_a, dtype=np.float32)
    a = np.asarray(a, dtype=np.float32)

    u = w @ (w_a @ a[2:4])  # [256, 1]
    # wext [256, 257] -> partition-major image [128, 2, 257]
    Mw = np.concatenate([w, u], axis=1).astype(ml_dtypes.bfloat16)
    pw_img = np.ascontiguousarray(Mw.reshape(2, P, FE).transpose(1, 0, 2))
    # node_feats^T [256, 8192] -> [128, 2, 8192], chunked along j
    nfT_img = node_feats.T.astype(ml_dtypes.bfloat16).reshape(2, P, N).transpose(1, 0, 2)
    nf_chunks = {
        f"nf{c}": np.ascontiguousarray(
            nfT_img[:, :, NSPLIT[c] * P : NSPLIT[c + 1] * P]
        )
        for c in range(len(NSPLIT) - 1)
    }

    in_maps = []
    for c in range(NCORES):
        aT_c = Ahat[c * ROWS : (c + 1) * ROWS, :].T  # [8192, 1024]
        aT_img = np.ascontiguousarray(
            aT_c.reshape(NJ, P, ROWS).transpose(1, 0, 2).astype(ml_dtypes.float8_e4m3)
        )
        in_maps.append({"aT": aT_img, "pw": pw_img, **nf_chunks})
    return in_maps


def _run(inputs, trace=False, **kwargs):
    if "nc" not in _CACHE:
        _CACHE["nc"] = _build()
    nc = _CACHE["nc"]
    in_maps = _prep_inputs(**inputs)
    res = run_bass_kernel_spmd(
        nc, in_maps, core_ids=list(range(NCORES)), trace=trace, **kwargs
    )
    # out image [128, 8, 256] -> rows (i*128 + p)
    full = np.concatenate(
        [
            res.results[c]["out"].transpose(1, 0, 2).reshape(ROWS, F)
            for c in range(NCORES)
        ],
        axis=0,
    )
    return full, res


def kernel(**inputs) -> np.ndarray:
    out, _ = _run(inputs, trace=False)
    return out
